# revision 1
# baseline (speedup 1.0000x reference)
"""ConcatCritic pair-grid MLP, v5: flipped mm2 layout + fused DVE reduce.

Per-core (64 rows of x, everything else replicated):
  setup:  transposes; mm1 -> hxbT[h, i](+b1) fp32, hyT[hb][h, j] fp16,
          w2T[hb][h, k] fp16 (W2 transposed), w3bc[j, k] = w3 replicated fp32
  per i:  ACT  A4[:, hb, :] = relu(hyT[hb] + hxbT[:, hb*64+i])   (4 ops, fp16)
          PE   pz[jc][j, k] += A4[:, hb, jc*128:+128].T @ w2T[hb]  (16 matmuls)
          DVE  acc[jc][:, i] = sum_k relu(pz[jc]) * w3bc          (4 fused ops)
  tail:   PE-transpose acc[jc] [j, i] -> [i, j]; +b3; one DMA out.

The W3 reduction costs zero PE time (fused into the DVE relu). b2 is
zero in this model family; a fallback build adds an exact K=1 matmul
(ones.T @ b2) into each psum accumulation when b2 != 0.
"""

import os

import numpy as np

import concourse.bass as bass
import concourse.bacc as bacc
import concourse.mybir as mybir
from concourse import tile
from concourse.masks import make_identity
from concourse.bass_utils import run_bass_kernel_spmd

B = 512
D = 128
H = 512
NCORES = 8
BI = B // NCORES  # 64 rows of x per core
HB = H // 128     # 4 h-blocks
JC = B // 128     # 4 j-chunks
FP = mybir.dt.float32
F16 = mybir.dt.float16

Relu = mybir.ActivationFunctionType.Relu
Identity = mybir.ActivationFunctionType.Identity
Add = mybir.AluOpType.add
Max = mybir.AluOpType.max
Mult = mybir.AluOpType.mult
Bypass = mybir.AluOpType.bypass


def build_v5(b2_nonzero: bool = False) -> bass.Bass:
    nc = bacc.Bacc(
        "TRN2",
        target_bir_lowering=False,
        debug=False,
        enable_asserts=False,
    )

    xs_d = nc.dram_tensor("xs", [BI, D], FP, kind="ExternalInput")
    y_d = nc.dram_tensor("y", [B, D], FP, kind="ExternalInput")
    W1_d = nc.dram_tensor("W1", [H, 2 * D], FP, kind="ExternalInput")
    b1_d = nc.dram_tensor("b1", [H], FP, kind="ExternalInput")
    W2_d = nc.dram_tensor("W2", [H, H], FP, kind="ExternalInput")
    b2_d = nc.dram_tensor("b2", [H], FP, kind="ExternalInput")
    W3_d = nc.dram_tensor("W3", [1, H], FP, kind="ExternalInput")
    b3_d = nc.dram_tensor("b3", [1], FP, kind="ExternalInput")
    out_d = nc.dram_tensor("out", [BI, B], FP, kind="ExternalOutput")

    with tile.TileContext(nc) as tc:
        with (
            tc.tile_pool(name="consts", bufs=1) as consts,
            tc.tile_pool(name="persist", bufs=1) as persist,
            tc.tile_pool(name="load", bufs=1) as load,
            tc.tile_pool(name="work", bufs=3) as work,
            tc.tile_pool(name="ps", bufs=8, space="PSUM") as ps,
        ):
            # ---------------- PE warm-up (overlaps input DMAs) ----------------
            warm_src = consts.tile([128, B], F16, name="warm_src")
            nc.vector.memset(warm_src, 0.0)
            warm_ps = ps.tile([128, B], FP, tag="misc", bufs=1, name="warm_ps")
            for _ in range(16):
                nc.tensor.matmul(
                    warm_ps, warm_src[:, :128], warm_src, start=True, stop=True
                )

            ident = consts.tile([128, 128], FP, name="ident")
            make_identity(nc, ident)

            # ---------------- input DMAs (4 queues) ----------------
            xs_sb = load.tile([BI, D], FP, name="xs_sb")
            y_sb = load.tile([128, B // 128, D], FP, name="y_sb")
            w1_sb = load.tile([128, HB, 2 * D], FP, name="w1_sb")
            w2_sb = load.tile([128, HB, H], FP, name="w2_sb")
            b1c = consts.tile([128, HB], FP, name="b1c")
            w3row = consts.tile([1, H], FP, name="w3row")
            b3c = consts.tile([1, 1], FP, name="b3c")
            nc.sync.dma_start(xs_sb, xs_d[:, :])
            nc.gpsimd.dma_start(y_sb, y_d[:].rearrange("(jb p) d -> p jb d", p=128))
            nc.scalar.dma_start(w1_sb, W1_d[:].rearrange("(hb p) d -> p hb d", p=128))
            for kb, eng in ((0, nc.sync), (2, nc.gpsimd), (1, nc.sync), (3, nc.gpsimd)):
                eng.dma_start(
                    w2_sb[:, kb : kb + 1, :],
                    W2_d[kb * 128 : (kb + 1) * 128].rearrange(
                        "(kb p) h -> p kb h", p=128
                    ),
                )
            nc.scalar.dma_start(b1c, b1_d[:].rearrange("(a p) -> p a", p=128))
            nc.scalar.dma_start(w3row, W3_d[:, :])
            nc.scalar.dma_start(b3c, b3_d[None, :])
            if b2_nonzero:
                b2row = consts.tile([1, H], F16, name="b2row")
                b2row32 = consts.tile([1, H], FP, name="b2row32")
                nc.scalar.dma_start(b2row32, b2_d[None, :])
                nc.vector.tensor_copy(b2row, b2row32)
                ones_st = consts.tile([1, 128], F16, name="ones_st")
                nc.vector.memset(ones_st, 1.0)

            ones1 = consts.tile([1, 128], FP, name="ones1")
            nc.vector.memset(ones1, 1.0)

            # ---------------- transposes + mm1 ----------------
            xsT = persist.tile([128, BI], FP, name="xsT")
            t_ps = ps.tile([128, 128], FP, tag="tbank", bufs=3, name="t_ps_x")
            nc.tensor.transpose(t_ps[:, :BI], xs_sb, ident[:BI, :BI])
            nc.vector.tensor_copy(xsT, t_ps[:, :BI])

            w1xT = []
            w1yT = []
            for hb in range(HB):
                tx = persist.tile([128, 128], FP, name=f"w1xT{hb}")
                ty = persist.tile([128, 128], FP, name=f"w1yT{hb}")
                px = ps.tile([128, 128], FP, tag="tbank", bufs=3, name=f"t_ps_w1x{hb}")
                nc.tensor.transpose(px, w1_sb[:, hb, :D], ident)
                nc.vector.tensor_copy(tx, px)
                py = ps.tile([128, 128], FP, tag="tbank", bufs=3, name=f"t_ps_w1y{hb}")
                nc.tensor.transpose(py, w1_sb[:, hb, D:], ident)
                nc.vector.tensor_copy(ty, py)
                w1xT.append(tx)
                w1yT.append(ty)

            yT = persist.tile([128, B], FP, name="yT")
            for jb in range(B // 128):
                pj = ps.tile([128, 128], FP, tag="tbank", bufs=3, name=f"t_ps_y{jb}")
                nc.tensor.transpose(pj, y_sb[:, jb, :], ident)
                nc.vector.tensor_copy(yT[:, jb * 128 : (jb + 1) * 128], pj)

            # hxbT[h, hb*BI + i] = (x @ W1x.T)[i, h] + b1[h]   (fp32)
            hxbT = persist.tile([128, HB * BI], FP, name="hxbT")
            hyT = [persist.tile([128, B], F16, name=f"hyT{hb}") for hb in range(HB)]
            for hb in range(HB):
                hx_ps = ps.tile([128, BI], FP, tag="tbank", bufs=3, name=f"hx_ps{hb}")
                nc.tensor.matmul(hx_ps, w1xT[hb], xsT, start=True, stop=True)
                nc.scalar.activation(
                    hxbT[:, hb * BI : (hb + 1) * BI],
                    hx_ps,
                    Identity,
                    bias=b1c[:, hb : hb + 1],
                )
                hy_ps = ps.tile([128, B], FP, tag="pz", bufs=4, name=f"hy_ps{hb}")
                nc.tensor.matmul(hy_ps, w1yT[hb], yT, start=True, stop=True)
                nc.scalar.activation(hyT[hb], hy_ps, Identity)

            # W2 transposes: w2T[hb][h', k] = W2[k, hb*128 + h']  (fp16)
            w2T = [persist.tile([128, H], F16, name=f"w2T{hb}") for hb in range(HB)]
            for kb in range(HB):
                for hb in range(HB):
                    pw = ps.tile(
                        [128, 128], FP, tag="tbank", bufs=3, name=f"t_ps_w2_{kb}_{hb}"
                    )
                    nc.tensor.transpose(
                        pw, w2_sb[:, kb, hb * 128 : (hb + 1) * 128], ident
                    )
                    dst = w2T[hb][:, kb * 128 : (kb + 1) * 128]
                    if (kb * HB + hb) % 2 == 0:
                        nc.vector.tensor_copy(dst, pw)
                    else:
                        nc.scalar.activation(dst, pw, Identity)

            # w3 broadcast to all 128 partitions: w3bc[j, k] = w3[k]
            w3bc_ps = ps.tile([128, B], FP, tag="misc", bufs=1, name="w3bc_ps")
            nc.tensor.matmul(w3bc_ps, ones1, w3row, start=True, stop=True)
            w3bc = consts.tile([128, B], FP, name="w3bc")
            nc.vector.tensor_copy(w3bc, w3bc_ps)

            # b3 broadcast to partitions (only first BI rows used at the tail)
            b3_ps = ps.tile([128, 1], FP, tag="misc", bufs=1, name="b3_ps")
            nc.tensor.matmul(b3_ps, ones1, b3c, start=True, stop=True)
            b3bc = consts.tile([128, 1], FP, name="b3bc")
            nc.vector.tensor_copy(b3bc, b3_ps)

            # accumulator staging: acc[jc][j, i] collects output columns
            acc = [persist.tile([128, BI], FP, name=f"acc{jc}") for jc in range(JC)]
            dummy = persist.tile([128, B], F16, name="stt_dummy")

            # ---------------- main loop ----------------
            def gen_A(i, A4):
                for hb in range(HB):
                    bias = hxbT[:, hb * BI + i : hb * BI + i + 1]
                    if i < 2 and hb < 2:
                        nc.vector.tensor_scalar(
                            A4[:, hb, :], hyT[hb], bias, 0.0, Add, Max
                        )
                    else:
                        nc.scalar.activation(A4[:, hb, :], hyT[hb], Relu, bias=bias)

            A_bufs = [
                work.tile([128, HB, B], F16, tag="A4", bufs=3, name=f"A4_{p}")
                for p in range(3)
            ]

            out_sb = persist.tile([BI, B], FP, name="out_sb")

            def stage_out(i0, i1):
                # acc[jc][:, i0:i1] ([j, i] cols) -> out rows i0:i1 (+ b3)
                n = i1 - i0
                for jc in range(JC):
                    pt = ps.tile([BI, 128], FP, tag="tbank", bufs=3, name=f"pt{i0}_{jc}")
                    nc.tensor.transpose(pt[:n, :], acc[jc][:, i0:i1], ident)
                    nc.vector.tensor_scalar(
                        out_sb[i0:i1, jc * 128 : (jc + 1) * 128],
                        pt[:n, :],
                        b3bc[:n, :],
                        0.0,
                        Add,
                        Bypass,
                    )
                nc.sync.dma_start(out_d[i0:i1, :], out_sb[i0:i1, :])

            gen_A(0, A_bufs[0])
            for i in range(BI):
                A4 = A_bufs[i % 3]
                if i + 1 < BI:
                    gen_A(i + 1, A_bufs[(i + 1) % 3])
                if i == 40:
                    stage_out(0, 32)
                for jc in range(JC):
                    pz = ps.tile(
                        [128, B], FP, tag="pz", bufs=4, name=f"pz{i}_{jc}"
                    )
                    if b2_nonzero:
                        nc.tensor.matmul(pz, ones_st, b2row, start=True, stop=False)
                    for hb in range(HB):
                        nc.tensor.matmul(
                            pz,
                            A4[:, hb, jc * 128 : (jc + 1) * 128],
                            w2T[hb],
                            start=(hb == 0 and not b2_nonzero),
                            stop=(hb == HB - 1),
                        )
                    # acc[jc][:, i] = sum_k relu(pz) * w3
                    nc.vector.scalar_tensor_tensor(
                        dummy,
                        pz,
                        0.0,
                        w3bc,
                        Max,
                        Mult,
                        accum_out=acc[jc][:, i : i + 1],
                    )
            stage_out(32, BI)

    nc.compile()
    return nc


_BUILT: dict[str, bass.Bass] = {}


def _get_nc(key: str) -> bass.Bass:
    if key not in _BUILT:
        _BUILT[key] = build_v5(b2_nonzero=(key == "b2"))
    return _BUILT[key]


def run(inputs: dict, variant: str | None = None, trace: bool = False):
    x = np.ascontiguousarray(np.asarray(inputs["x"], dtype=np.float32))
    y = np.ascontiguousarray(np.asarray(inputs["y"], dtype=np.float32))
    W1 = np.ascontiguousarray(np.asarray(inputs["W1"], dtype=np.float32))
    b1 = np.ascontiguousarray(np.asarray(inputs["b1"], dtype=np.float32))
    W2 = np.ascontiguousarray(np.asarray(inputs["W2"], dtype=np.float32))
    b2 = np.ascontiguousarray(np.asarray(inputs["b2"], dtype=np.float32))
    W3 = np.ascontiguousarray(np.asarray(inputs["W3"], dtype=np.float32))
    b3 = np.ascontiguousarray(np.asarray(inputs["b3"], dtype=np.float32))
    nc = _get_nc("b2" if np.any(b2) else "z")
    in_maps = []
    for c in range(NCORES):
        in_maps.append(
            {
                "xs": np.ascontiguousarray(x[c * BI : (c + 1) * BI]),
                "y": y,
                "W1": W1,
                "b1": b1,
                "W2": W2,
                "b2": b2,
                "W3": W3,
                "b3": b3,
            }
        )
    res = run_bass_kernel_spmd(nc, in_maps, core_ids=list(range(NCORES)), trace=trace)
    out = np.concatenate([r["out"] for r in res.results], axis=0)
    return out, res


def kernel(**inputs) -> np.ndarray:
    out, _ = run(inputs)
    return out

